# revision 30
# baseline (speedup 1.0000x reference)
"""Trainium2 Bass kernel for nn_ConsistencyLoss.

loss = -mean(masked_select(cos_sim(mom[b,:,m], base[b,:,n]), mask))

Reformulation so the 268MB int32 mask is streamed once:

    masked_sum = sum_{b,m,n} M[b,m,n] * (mhat_m . bhat_n)
               = sum_{b,c,n} bhat[c,n] * W[c,n],   W = mhatT.T @ M   (contract over m)
    loss       = -masked_sum / sum(M)

Sharding: 8 cores = (batch b in 0..3) x (half of the m rows).

v11 layout per core (PSUM used only by the 8 W banks, so reps overlap):
  - mom f32->fp16 cast-DMA; 16 XBAR DMA-transposes (SP queue) -> momT in
    SBUF; batched square/reduce/rsqrt chain -> mh bf16 (single-pass lhs,
    ~1e-3 err vs the 2e-2 gate)
  - base f32->fp16 cast-DMA; bsq = base*base (DVE 2x); nb2 via gpsimd
    partition_all_reduce; rsqrt chain; bhat = base * rnb (fp16)
  - mask: 8 cast-DMAs int32->bf16, one per [128, 2, 4096] tile pair
    (the stream is read-side limited on HW, so bf16 out is free and
    enables the fast DVE count mode)
  - W accumulated in 8 PSUM banks with bf16 x bf16 matmuls
  - count fused into streaming: DVE tensor_scalar(accum_out) for 5 pairs,
    Act activation(Square, accum_out) for 3
  - tail: per-bank Q = W * bhat (DVE, PSUM read) + Act Copy-accum -> scol;
    final (sum, count) pair all-reduced across partitions on gpsimd
Host combines 8 (sum, count) pairs.
"""

import sys

for _p in ("/opt/trn_rl_repo",):
    if _p not in sys.path:
        sys.path.insert(0, _p)

import numpy as np

B, C, HW = 4, 128, 4096          # batch, channels, H*W
M_LOC = HW // 2                  # momentum rows per core
N_CORES = 8

# count-engine schedule per mask tile PAIR: D=vector(tensor_scalar), A=scalar(act)
COUNT_ASSIGN = "DDDDDAAA"
# engines for the 16 mh scale ops / 8 bhat muls: "pool" offloads them to gpsimd
MH_ENGINE = "vector"
BHAT_ENGINE = "vector"

_RUNNER = None


def build_nc(n=HW, m_loc=M_LOC, n_cores=N_CORES, reps=1,
             count_assign=COUNT_ASSIGN, mh_engine=MH_ENGINE,
             bhat_engine=BHAT_ENGINE):
    """Build the per-core Bass module. n = base pixels, m_loc = local momentum rows."""
    import concourse.bass as bass
    import concourse.mybir as mybir
    import concourse.tile as tile
    from concourse import bacc
    from concourse.masks import make_identity
    from concourse.bass import ts

    f32 = mybir.dt.float32
    f32r = mybir.dt.float32r
    bf16 = mybir.dt.bfloat16
    fp16 = mybir.dt.float16
    fp8 = mybir.dt.float8e4
    i32 = mybir.dt.int32
    AX = mybir.AxisListType
    ALU = mybir.AluOpType
    ACT = mybir.ActivationFunctionType
    T = m_loc // 128             # mask row tiles
    U = T // 2                   # mask tile pairs (one cast-DMA each)
    NB = n // 512                # 512-wide n blocks (one PSUM bank each)
    assert NB <= 8 and U == len(count_assign)

    nc = bacc.Bacc("TRN2", target_bir_lowering=False, debug=False,
                   num_devices=n_cores)
    base_d = nc.declare_dram_parameter("base", [128, n], f32, isOutput=False)
    mom_d = nc.declare_dram_parameter("mom", [128, m_loc], f32, isOutput=False)
    mask_d = nc.declare_dram_parameter("mask", [m_loc, n], i32, isOutput=False)
    out_d = nc.declare_dram_parameter("out", [1, 2 * reps], f32, isOutput=True)
    maskP = mask_d.rearrange("(u kt p) n -> u p kt n", p=128, kt=2)

    with tile.TileContext(nc) as tc:
        import concourse.bass_isa as bass_isa
        RED = bass_isa.ReduceOp
        from contextlib import ExitStack
        with ExitStack() as ctx:
            sb = ctx.enter_context(tc.tile_pool(name="sb", bufs=1))
            mkp = ctx.enter_context(tc.tile_pool(name="mk", bufs=1))
            scrp = ctx.enter_context(tc.tile_pool(name="scr", bufs=1))

            outpair = sb.tile([1, 2 * reps], f32)

            for rep in range(reps):
                # ---- all DMAs issued upfront (SWDGE casts; 8 pair buffers
                # so no dma_start head-blocks) ----
                sb_mom = sb.tile([128, m_loc], fp16, tag="sb_mom")
                nc.gpsimd.dma_start(out=sb_mom[:], in_=mom_d[:])
                sb_base = sb.tile([128, n], fp16, tag="sb_base")
                nc.gpsimd.dma_start(out=sb_base[:], in_=base_d[:])
                mks = []
                for u in range(U):
                    mk = mkp.tile([128, 2, n], bf16, tag=f"mk{u % 6}")
                    nc.gpsimd.dma_start(out=mk[:], in_=maskP[u])
                    mks.append(mk)

                scrD = sb.tile([128, 2 * n], bf16, tag="scrD")
                cntc = sb.tile([128, U], f32, tag="cntc")
                scol = sb.tile([128, NB], f32, tag="scol")
                pairT = sb.tile([128, 2], f32, tag="pairT")
                pairAll = sb.tile([128, 2], f32, tag="pairAll")
                mh = sb.tile([128, T, 128], bf16, tag="mh")
                momT = sb.tile([128, T, 128], fp16, tag="momT")
                nsq = sb.tile([128, T], f32, tag="nsq")
                rn_a = sb.tile([128, T], f32, tag="rn_a")
                rn = sb.tile([128, T], f32, tag="rn")
                bhat = sb.tile([128, n], fp16, tag="bhat")
                b16 = sb.tile([128, n], fp16, tag="b16")
                scrQ8 = sb.tile([128, n], f32, tag="scrQ8")
                bsq = scrp.tile([128, n], fp16, tag="bsq")

                # ---- phase B part 1: bsq on DVE (2x fp16), ready early ----
                nc.vector.tensor_mul(bsq[:], sb_base[:], sb_base[:])

                # ---- phase A: XBAR DMA-transposes (SP queue, no PSUM) ----
                for t in range(T):
                    nc.sync.dma_start(out=momT[:, t, :],
                                      in_=sb_mom[:, ts(t, 128)], transpose=True)
                sqA = scrp.tile([128, m_loc], fp16, tag="sqA")
                nc.vector.tensor_mul(
                    sqA[:], momT.rearrange("p t c -> p (t c)"),
                    momT.rearrange("p t c -> p (t c)"))
                nc.vector.reduce_sum(
                    out=nsq[:], in_=sqA.rearrange("p (t c) -> p t c", c=128),
                    axis=AX.X)
                nc.vector.reciprocal(out=rn_a[:], in_=nsq[:])
                nc.scalar.activation(out=rn[:], in_=rn_a[:], func=ACT.Sqrt)
                for t in range(T):
                    nc.vector.tensor_scalar_mul(
                        mh[:, t, :], momT[:, t, :], rn[:, t:t + 1])

                # ---- counts: after phase work so the tiny normalize ops
                # are not stuck behind 7us count ops ----
                for u in range(U):
                    if count_assign[u] == "D":
                        nc.vector.tensor_scalar(
                            out=scrD.rearrange("p (k n) -> p k n", k=2),
                            in0=mks[u][:], scalar1=1.0, scalar2=0.0,
                            op0=ALU.mult, op1=ALU.add,
                            accum_out=cntc[:, u:u + 1])
                    else:
                        nc.scalar.activation(
                            out=scrD.rearrange("p (k n) -> p k n", k=2),
                            in_=mks[u][:],
                            func=ACT.Square,
                            accum_out=cntc[:, u:u + 1])

                # ---- phase B part 2: nb2 all-reduce, rsqrt, bhat ----
                nc.gpsimd.partition_all_reduce(scrQ8[:], bsq[:], channels=128,
                                               reduce_op=RED.add)
                with nc.allow_low_precision("1/||b||^2 in fp16 is plenty"):
                    nc.vector.reciprocal(out=b16[:], in_=scrQ8[:])
                nc.scalar.activation(out=bsq[:], in_=b16[:], func=ACT.Sqrt)
                nc.vector.tensor_mul(bhat[:], sb_base[:], bsq[:])

                # ---- phase 1: W accumulation over mask pairs ----
                with tc.tile_pool(name=f"psW_{rep}", bufs=1, space="PSUM") as psW:
                    Wb = [psW.tile([128, 512], f32, tag=f"w{nb}", name=f"w{nb}")
                          for nb in range(NB)]
                    for u in range(U):
                        for kt in range(2):
                            for nb in range(NB):
                                nc.tensor.matmul(
                                    Wb[nb][:], mh[:, 2 * u + kt, :],
                                    mks[u][:, kt, ts(nb, 512)],
                                    start=(u == 0 and kt == 0),
                                    stop=(u == U - 1 and kt == 1))

                    # ---- tail: per-bank Q = W * bhat (DVE), accum on Act ----
                    for nb in range(NB):
                        nc.vector.tensor_mul(scrQ8[:, ts(nb, 512)],
                                             Wb[nb][:], bhat[:, ts(nb, 512)])
                        nc.scalar.activation(
                            out=scrD[:, ts(nb, 512)],
                            in_=scrQ8[:, ts(nb, 512)], func=ACT.Copy,
                            accum_out=scol[:, nb:nb + 1])

                # ---- finals: (masked_sum, count) per partition, all-reduce ----
                nc.vector.reduce_sum(out=pairT[:, 0:1], in_=scol[:], axis=AX.X)
                nc.vector.reduce_sum(out=pairT[:, 1:2], in_=cntc[:], axis=AX.X)
                nc.gpsimd.partition_all_reduce(pairAll[:], pairT[:],
                                               channels=128, reduce_op=RED.add)
                nc.vector.tensor_copy(
                    out=outpair[0:1, 2 * rep:2 * rep + 2], in_=pairAll[0:1, :])

            nc.sync.dma_start(out=out_d[:], in_=outpair[:])

    nc.compile()
    return nc


class SpmdRunner:
    """Compile-once PJRT runner; keeps staged inputs on device."""

    def __init__(self, nc, n_cores):
        import jax
        from jax.sharding import Mesh, PartitionSpec
        from jax.experimental.shard_map import shard_map
        import concourse.mybir as mybir
        from concourse.bass2jax import (_bass_exec_p, install_neuronx_cc_hook,
                                        partition_id_tensor)
        install_neuronx_cc_hook()
        self.jax = jax
        self.PartitionSpec = PartitionSpec
        self.n_cores = n_cores
        in_names, out_names, out_avals, zero_outs = [], [], [], []
        partition_name = (nc.partition_id_tensor.name
                          if nc.partition_id_tensor else None)
        for alloc in nc.m.functions[0].allocations:
            if not isinstance(alloc, mybir.MemoryLocationSet):
                continue
            name = alloc.memorylocations[0].name
            if alloc.kind == "ExternalInput":
                if name != partition_name:
                    in_names.append(name)
            elif alloc.kind == "ExternalOutput":
                out_names.append(name)
                shape = tuple(alloc.tensor_shape)
                dtype = mybir.dt.np(alloc.dtype)
                out_avals.append(jax.core.ShapedArray(shape, dtype))
                zero_outs.append(np.zeros(shape, dtype))
        self.in_names, self.out_names = in_names, out_names
        self.zero_outs = zero_outs
        n_params = len(in_names)
        all_in_names = in_names + out_names
        if partition_name is not None:
            all_in_names.append(partition_name)

        def _body(*args):
            operands = list(args)
            if partition_name is not None:
                operands.append(partition_id_tensor())
            outs = _bass_exec_p.bind(
                *operands,
                out_avals=tuple(out_avals),
                in_names=tuple(all_in_names),
                out_names=tuple(out_names),
                lowering_input_output_aliases=(),
                sim_require_finite=True,
                sim_require_nnan=True,
                nc=nc,
            )
            return tuple(outs)

        devices = jax.devices()[:n_cores]
        self.mesh = Mesh(np.asarray(devices), ("core",))
        in_specs = (PartitionSpec("core"),) * (n_params + len(out_names))
        out_specs = (PartitionSpec("core"),) * len(out_names)
        self.fn = jax.jit(shard_map(_body, mesh=self.mesh, in_specs=in_specs,
                                    out_specs=out_specs, check_rep=False))

    def stage(self, in_maps):
        from jax.sharding import NamedSharding
        args = []
        for name in self.in_names:
            glob = np.concatenate([np.asarray(m[name]) for m in in_maps], axis=0)
            args.append(self.jax.device_put(
                glob, NamedSharding(self.mesh, self.PartitionSpec("core"))))
        for z in self.zero_outs:
            glob = np.concatenate([z] * self.n_cores, axis=0)
            args.append(self.jax.device_put(
                glob, NamedSharding(self.mesh, self.PartitionSpec("core"))))
        return args

    def run(self, args):
        outs = self.fn(*args)
        self.jax.block_until_ready(outs)
        return outs

    def results(self, outs):
        res = [dict() for _ in range(self.n_cores)]
        for i, name in enumerate(self.out_names):
            glob = np.asarray(outs[i])
            per = np.split(glob, self.n_cores, axis=0)
            for c in range(self.n_cores):
                res[c][name] = per[c]
        return res


def make_in_maps(en_base, en_momentum, matrix):
    """Slice full inputs per core: core k -> (batch k//2, m-half k%2)."""
    in_maps = []
    for k in range(N_CORES):
        b, h = k // 2, k % 2
        base = np.ascontiguousarray(en_base[b].reshape(C, HW))
        mom = np.ascontiguousarray(
            en_momentum[b].reshape(C, HW)[:, h * M_LOC:(h + 1) * M_LOC])
        mask = matrix[b, h * M_LOC:(h + 1) * M_LOC, :]
        in_maps.append({"base": base, "mom": mom, "mask": mask})
    return in_maps


def _get_runner():
    global _RUNNER
    if _RUNNER is None:
        nc = build_nc()
        _RUNNER = SpmdRunner(nc, N_CORES)
    return _RUNNER


def kernel(en_base, en_momentum, matrix):
    runner = _get_runner()
    args = runner.stage(make_in_maps(en_base, en_momentum, matrix))
    res = runner.results(runner.run(args))
    tot = np.zeros(2, dtype=np.float64)
    for c in range(N_CORES):
        tot += res[c]["out"][0, :2].astype(np.float64)
    loss = -(tot[0] / tot[1])
    return np.array(loss, dtype=np.float32)


# revision 31
# speedup vs baseline: 1.0067x; 1.0067x over previous
"""Trainium2 Bass kernel for nn_ConsistencyLoss.

loss = -mean(masked_select(cos_sim(mom[b,:,m], base[b,:,n]), mask))

Reformulation so the 268MB int32 mask is streamed once:

    masked_sum = sum_{b,m,n} M[b,m,n] * (mhat_m . bhat_n)
               = sum_{b,c,n} bhat[c,n] * W[c,n],   W = mhatT.T @ M   (contract over m)
    loss       = -masked_sum / sum(M)

Sharding: 8 cores = (batch b in 0..3) x (half of the m rows).

v11 layout per core (PSUM used only by the 8 W banks, so reps overlap):
  - mom f32->fp16 cast-DMA; 16 XBAR DMA-transposes (SP queue) -> momT in
    SBUF; batched square/reduce/rsqrt chain -> mh bf16 (single-pass lhs,
    ~1e-3 err vs the 2e-2 gate)
  - base f32->fp16 cast-DMA; bsq = base*base (DVE 2x); nb2 via gpsimd
    partition_all_reduce; rsqrt chain; bhat = base * rnb (fp16)
  - mask: 8 cast-DMAs int32->bf16, one per [128, 2, 4096] tile pair
    (the stream is read-side limited on HW, so bf16 out is free and
    enables the fast DVE count mode)
  - W accumulated in 8 PSUM banks with bf16 x bf16 matmuls
  - count fused into streaming: DVE tensor_scalar(accum_out) for 5 pairs,
    Act activation(Square, accum_out) for 3
  - tail: per-bank Q = W * bhat (DVE, PSUM read) + Act Copy-accum -> scol;
    final (sum, count) pair all-reduced across partitions on gpsimd
Host combines 8 (sum, count) pairs.
"""

import sys

for _p in ("/opt/trn_rl_repo",):
    if _p not in sys.path:
        sys.path.insert(0, _p)

import numpy as np

B, C, HW = 4, 128, 4096          # batch, channels, H*W
M_LOC = HW // 2                  # momentum rows per core
N_CORES = 8

# count-engine schedule per mask tile PAIR: D=vector(tensor_scalar), A=scalar(act)
COUNT_ASSIGN = "DDAAADDD"
# engines for the 16 mh scale ops / 8 bhat muls: "pool" offloads them to gpsimd
MH_ENGINE = "vector"
BHAT_ENGINE = "vector"

_RUNNER = None


def build_nc(n=HW, m_loc=M_LOC, n_cores=N_CORES, reps=1,
             count_assign=COUNT_ASSIGN, mh_engine=MH_ENGINE,
             bhat_engine=BHAT_ENGINE):
    """Build the per-core Bass module. n = base pixels, m_loc = local momentum rows."""
    import concourse.bass as bass
    import concourse.mybir as mybir
    import concourse.tile as tile
    from concourse import bacc
    from concourse.masks import make_identity
    from concourse.bass import ts

    f32 = mybir.dt.float32
    f32r = mybir.dt.float32r
    bf16 = mybir.dt.bfloat16
    fp16 = mybir.dt.float16
    fp8 = mybir.dt.float8e4
    i32 = mybir.dt.int32
    AX = mybir.AxisListType
    ALU = mybir.AluOpType
    ACT = mybir.ActivationFunctionType
    T = m_loc // 128             # mask row tiles
    U = T // 2                   # mask tile pairs (one cast-DMA each)
    NB = n // 512                # 512-wide n blocks (one PSUM bank each)
    assert NB <= 8 and U == len(count_assign)

    nc = bacc.Bacc("TRN2", target_bir_lowering=False, debug=False,
                   num_devices=n_cores)
    base_d = nc.declare_dram_parameter("base", [128, n], f32, isOutput=False)
    mom_d = nc.declare_dram_parameter("mom", [128, m_loc], f32, isOutput=False)
    mask_d = nc.declare_dram_parameter("mask", [m_loc, n], i32, isOutput=False)
    out_d = nc.declare_dram_parameter("out", [1, 2 * reps], f32, isOutput=True)
    maskP = mask_d.rearrange("(u kt p) n -> u p kt n", p=128, kt=2)

    with tile.TileContext(nc) as tc:
        import concourse.bass_isa as bass_isa
        RED = bass_isa.ReduceOp
        from contextlib import ExitStack
        with ExitStack() as ctx:
            sb = ctx.enter_context(tc.tile_pool(name="sb", bufs=1))
            mkp = ctx.enter_context(tc.tile_pool(name="mk", bufs=1))
            scrp = ctx.enter_context(tc.tile_pool(name="scr", bufs=1))

            outpair = sb.tile([1, 2 * reps], f32)

            for rep in range(reps):
                # ---- all DMAs issued upfront (SWDGE casts; 8 pair buffers
                # so no dma_start head-blocks) ----
                sb_mom = sb.tile([128, m_loc], fp16, tag="sb_mom")
                nc.gpsimd.dma_start(out=sb_mom[:], in_=mom_d[:])
                sb_base = sb.tile([128, n], fp16, tag="sb_base")
                nc.gpsimd.dma_start(out=sb_base[:], in_=base_d[:])
                mks = []
                for u in range(U):
                    mk = mkp.tile([128, 2, n], bf16, tag=f"mk{u % 6}")
                    nc.gpsimd.dma_start(out=mk[:], in_=maskP[u])
                    mks.append(mk)

                scrD = sb.tile([128, 2 * n], bf16, tag="scrD")
                cntc = sb.tile([128, U], f32, tag="cntc")
                scol = sb.tile([128, NB], f32, tag="scol")
                pairT = sb.tile([128, 2], f32, tag="pairT")
                pairAll = sb.tile([128, 2], f32, tag="pairAll")
                mh = sb.tile([128, T, 128], bf16, tag="mh")
                momT = sb.tile([128, T, 128], fp16, tag="momT")
                nsq = sb.tile([128, T], f32, tag="nsq")
                rn_a = sb.tile([128, T], f32, tag="rn_a")
                rn = sb.tile([128, T], f32, tag="rn")
                bhat = sb.tile([128, n], fp16, tag="bhat")
                b16 = sb.tile([128, n], fp16, tag="b16")
                scrQ8 = sb.tile([128, n], f32, tag="scrQ8")
                bsq = scrp.tile([128, n], fp16, tag="bsq")

                # ---- phase B part 1: bsq on DVE (2x fp16), ready early ----
                nc.vector.tensor_mul(bsq[:], sb_base[:], sb_base[:])

                # ---- phase A: XBAR DMA-transposes (SP queue, no PSUM) ----
                for t in range(T):
                    nc.sync.dma_start(out=momT[:, t, :],
                                      in_=sb_mom[:, ts(t, 128)], transpose=True)
                sqA = scrp.tile([128, m_loc], fp16, tag="sqA")
                nc.vector.tensor_mul(
                    sqA[:], momT.rearrange("p t c -> p (t c)"),
                    momT.rearrange("p t c -> p (t c)"))
                nc.vector.reduce_sum(
                    out=nsq[:], in_=sqA.rearrange("p (t c) -> p t c", c=128),
                    axis=AX.X)
                nc.vector.reciprocal(out=rn_a[:], in_=nsq[:])
                nc.scalar.activation(out=rn[:], in_=rn_a[:], func=ACT.Sqrt)
                for t in range(T):
                    nc.vector.tensor_scalar_mul(
                        mh[:, t, :], momT[:, t, :], rn[:, t:t + 1])

                # ---- counts: after phase work so the tiny normalize ops
                # are not stuck behind 7us count ops ----
                for u in range(U):
                    if count_assign[u] == "D":
                        nc.vector.tensor_scalar(
                            out=scrD.rearrange("p (k n) -> p k n", k=2),
                            in0=mks[u][:], scalar1=1.0, scalar2=0.0,
                            op0=ALU.mult, op1=ALU.add,
                            accum_out=cntc[:, u:u + 1])
                    else:
                        nc.scalar.activation(
                            out=scrD.rearrange("p (k n) -> p k n", k=2),
                            in_=mks[u][:],
                            func=ACT.Square,
                            accum_out=cntc[:, u:u + 1])

                # ---- phase B part 2: nb2 all-reduce, rsqrt, bhat ----
                nc.gpsimd.partition_all_reduce(scrQ8[:], bsq[:], channels=128,
                                               reduce_op=RED.add)
                with nc.allow_low_precision("1/||b||^2 in fp16 is plenty"):
                    nc.vector.reciprocal(out=b16[:], in_=scrQ8[:])
                nc.scalar.activation(out=bsq[:], in_=b16[:], func=ACT.Sqrt)
                nc.vector.tensor_mul(bhat[:], sb_base[:], bsq[:])

                # ---- phase 1: W accumulation over mask pairs ----
                with tc.tile_pool(name=f"psW_{rep}", bufs=1, space="PSUM") as psW:
                    Wb = [psW.tile([128, 512], f32, tag=f"w{nb}", name=f"w{nb}")
                          for nb in range(NB)]
                    for u in range(U):
                        for kt in range(2):
                            for nb in range(NB):
                                nc.tensor.matmul(
                                    Wb[nb][:], mh[:, 2 * u + kt, :],
                                    mks[u][:, kt, ts(nb, 512)],
                                    start=(u == 0 and kt == 0),
                                    stop=(u == U - 1 and kt == 1))

                    # ---- tail: per-bank Q = W * bhat (DVE), accum on Act ----
                    for nb in range(NB):
                        nc.vector.tensor_mul(scrQ8[:, ts(nb, 512)],
                                             Wb[nb][:], bhat[:, ts(nb, 512)])
                        nc.scalar.activation(
                            out=scrD[:, ts(nb, 512)],
                            in_=scrQ8[:, ts(nb, 512)], func=ACT.Copy,
                            accum_out=scol[:, nb:nb + 1])

                # ---- finals: (masked_sum, count) per partition, all-reduce ----
                nc.vector.reduce_sum(out=pairT[:, 0:1], in_=scol[:], axis=AX.X)
                nc.vector.reduce_sum(out=pairT[:, 1:2], in_=cntc[:], axis=AX.X)
                nc.gpsimd.partition_all_reduce(pairAll[:], pairT[:],
                                               channels=128, reduce_op=RED.add)
                nc.vector.tensor_copy(
                    out=outpair[0:1, 2 * rep:2 * rep + 2], in_=pairAll[0:1, :])

            nc.sync.dma_start(out=out_d[:], in_=outpair[:])

    nc.compile()
    return nc


class SpmdRunner:
    """Compile-once PJRT runner; keeps staged inputs on device."""

    def __init__(self, nc, n_cores):
        import jax
        from jax.sharding import Mesh, PartitionSpec
        from jax.experimental.shard_map import shard_map
        import concourse.mybir as mybir
        from concourse.bass2jax import (_bass_exec_p, install_neuronx_cc_hook,
                                        partition_id_tensor)
        install_neuronx_cc_hook()
        self.jax = jax
        self.PartitionSpec = PartitionSpec
        self.n_cores = n_cores
        in_names, out_names, out_avals, zero_outs = [], [], [], []
        partition_name = (nc.partition_id_tensor.name
                          if nc.partition_id_tensor else None)
        for alloc in nc.m.functions[0].allocations:
            if not isinstance(alloc, mybir.MemoryLocationSet):
                continue
            name = alloc.memorylocations[0].name
            if alloc.kind == "ExternalInput":
                if name != partition_name:
                    in_names.append(name)
            elif alloc.kind == "ExternalOutput":
                out_names.append(name)
                shape = tuple(alloc.tensor_shape)
                dtype = mybir.dt.np(alloc.dtype)
                out_avals.append(jax.core.ShapedArray(shape, dtype))
                zero_outs.append(np.zeros(shape, dtype))
        self.in_names, self.out_names = in_names, out_names
        self.zero_outs = zero_outs
        n_params = len(in_names)
        all_in_names = in_names + out_names
        if partition_name is not None:
            all_in_names.append(partition_name)

        def _body(*args):
            operands = list(args)
            if partition_name is not None:
                operands.append(partition_id_tensor())
            outs = _bass_exec_p.bind(
                *operands,
                out_avals=tuple(out_avals),
                in_names=tuple(all_in_names),
                out_names=tuple(out_names),
                lowering_input_output_aliases=(),
                sim_require_finite=True,
                sim_require_nnan=True,
                nc=nc,
            )
            return tuple(outs)

        devices = jax.devices()[:n_cores]
        self.mesh = Mesh(np.asarray(devices), ("core",))
        in_specs = (PartitionSpec("core"),) * (n_params + len(out_names))
        out_specs = (PartitionSpec("core"),) * len(out_names)
        self.fn = jax.jit(shard_map(_body, mesh=self.mesh, in_specs=in_specs,
                                    out_specs=out_specs, check_rep=False))

    def stage(self, in_maps):
        from jax.sharding import NamedSharding
        args = []
        for name in self.in_names:
            glob = np.concatenate([np.asarray(m[name]) for m in in_maps], axis=0)
            args.append(self.jax.device_put(
                glob, NamedSharding(self.mesh, self.PartitionSpec("core"))))
        for z in self.zero_outs:
            glob = np.concatenate([z] * self.n_cores, axis=0)
            args.append(self.jax.device_put(
                glob, NamedSharding(self.mesh, self.PartitionSpec("core"))))
        return args

    def run(self, args):
        outs = self.fn(*args)
        self.jax.block_until_ready(outs)
        return outs

    def results(self, outs):
        res = [dict() for _ in range(self.n_cores)]
        for i, name in enumerate(self.out_names):
            glob = np.asarray(outs[i])
            per = np.split(glob, self.n_cores, axis=0)
            for c in range(self.n_cores):
                res[c][name] = per[c]
        return res


def make_in_maps(en_base, en_momentum, matrix):
    """Slice full inputs per core: core k -> (batch k//2, m-half k%2)."""
    in_maps = []
    for k in range(N_CORES):
        b, h = k // 2, k % 2
        base = np.ascontiguousarray(en_base[b].reshape(C, HW))
        mom = np.ascontiguousarray(
            en_momentum[b].reshape(C, HW)[:, h * M_LOC:(h + 1) * M_LOC])
        mask = matrix[b, h * M_LOC:(h + 1) * M_LOC, :]
        in_maps.append({"base": base, "mom": mom, "mask": mask})
    return in_maps


def _get_runner():
    global _RUNNER
    if _RUNNER is None:
        nc = build_nc()
        _RUNNER = SpmdRunner(nc, N_CORES)
    return _RUNNER


def kernel(en_base, en_momentum, matrix):
    runner = _get_runner()
    args = runner.stage(make_in_maps(en_base, en_momentum, matrix))
    res = runner.results(runner.run(args))
    tot = np.zeros(2, dtype=np.float64)
    for c in range(N_CORES):
        tot += res[c]["out"][0, :2].astype(np.float64)
    loss = -(tot[0] / tot[1])
    return np.array(loss, dtype=np.float32)
